# revision 1
# baseline (speedup 1.0000x reference)
"""Trainium2 Bass kernel for nn_MetricSelfAttention.

Reference computation (B=4, W=2048, C=1024, N=16 heads, K=64):
    metric_n = P_n @ P_n^T                  (per-head bilinear form)
    proj = X @ W_proj^T ; split into per-head Q_n [W, K]
    S_n = tril(Q_n M_n Q_n^T) / sqrt(K)     (multiplicative causal mask, no softmax)
    U_n = S_n @ Q_n
    out = concat_n(U_n @ T_n) @ W_mixer^T

Device algorithm (per core; 8 cores = 4 batches x 2 head-groups of 8 heads):
  Host folds:  M'_n = P_n P_n^T / sqrt(K),  Wm2_n = T_n @ W_mixer[:, nK:(n+1)K]^T
  so that out_partial = sum_n U_n @ Wm2_n with U_n = tril(Q_n M'_n Q_n^T) @ Q_n.

  Causal decomposition (block row i of 128):
    U_i = A_i @ KV_i + tril(A_i Q_i^T) @ Q_i,   A = Q M',  KV_i = sum_{j<i} Q_j^T Q_j

  Structure (199us baseline -> ~126us):
   - the transposed projection QT is obtained by PE transposes of the natural
     projection (8K PE rows) instead of a full second projection pass (64K rows)
   - PSUM drains are batched into [128,512] ops and split across DVE/Scalar
     (GpSimd cannot touch PSUM); the KV prefix is 4 per-pair bf16 add chains
     on GpSimd over pre-masked gram terms, off the PE critical path
   - phase 1 trails the transposes/gram TWO iterations behind the projection
     so their weight loads never stall on the vector q_nat drain; phase 2/3
     pipelines per w-tile with D1a two tiles AHEAD and the mixer one tile
     BEHIND D1b, and psM=3 lets mixer chunks reuse buffers drained 1.5
     iterations earlier
   - input descriptors are spread across the three DMA-capable queues
     (sync/scalar/gpsimd, ~650ns issue each); output is bf16, streamed per
     512-chunk; partials are summed in f32 on the host
   - fp8e4 DoubleRow was tried and REVERTED: on this hardware a DR matmul
     streams output columns at the same rate as bf16 (no 0.5 cycles/row),
     so the 3-term residual split needed for accuracy costs 1.5x bf16
"""

import os
import sys

import numpy as np
import ml_dtypes

if "/opt/trn_rl_repo" not in sys.path:
    sys.path.insert(0, "/opt/trn_rl_repo")

import concourse.bass as bass
import concourse.tile as tile
from concourse import bacc, mybir
from concourse.bass_utils import run_bass_kernel_spmd

BF16 = mybir.dt.bfloat16
F32 = mybir.dt.float32

B, W, C, NHEADS, K = 4, 2048, 1024, 16, 64
HPG = 8          # heads per group (per core)
NPAIR = 4        # head pairs per core
GK = HPG * K     # 512: head-group projection width

_NC_CACHE = {}
LAST_RESULTS = None  # for test.py introspection (exec_time_ns etc.)


def build_nc(w=W, mm_dt=BF16):
    """Build the per-core Bass program. Parameterized by sequence length for
    small-scale simulator testing."""
    nw = w // 128           # number of 128-row w-tiles
    csub = C // 128         # 8 contraction subtiles for the projections
    chunk = min(512, w)
    nch = w // chunk        # 512-wide chunks of the sequence dim

    nc = bacc.Bacc()
    xt_d = nc.declare_dram_parameter("xt", [C, w], mm_dt, isOutput=False)
    wpt_d = nc.declare_dram_parameter("wpt", [C, GK], mm_dt, isOutput=False)
    mblk_d = nc.declare_dram_parameter("mblk", [NPAIR, 128, 128], mm_dt, isOutput=False)
    wm2_d = nc.declare_dram_parameter("wm2", [NPAIR, 128, C], mm_dt, isOutput=False)
    triu4_d = nc.declare_dram_parameter("triu4", [128, 512], F32, isOutput=False)
    blkd4_d = nc.declare_dram_parameter("blkd4", [128, 512], F32, isOutput=False)
    ident_d = nc.declare_dram_parameter("ident", [128, 128], mm_dt, isOutput=False)
    out_d = nc.declare_dram_parameter("out", [w, C], mm_dt, isOutput=True)

    from contextlib import ExitStack

    with tile.TileContext(nc) as tc, ExitStack() as ctx:
        const = ctx.enter_context(tc.tile_pool(name="const", bufs=1))
        persist = ctx.enter_context(tc.tile_pool(name="persist", bufs=1))

        # ---- constant / input loads ----
        # Descriptor issue is ~650ns/op per sequencer, and phase A's first
        # tile needs chunk 0 of every xt subtile plus all of wpt.  Only
        # sync/scalar/gpsimd can issue DMAs, so the first-tile descriptors
        # are spread over all three: sync 6 xt + gpsimd 6 wpt, scalar the
        # remaining 2+2 (interleaved in consumption order).
        wpt_sb = [const.tile([128, GK], mm_dt, name=f"wpt{s}", tag=f"wpt{s}")
                  for s in range(csub)]
        xt_sb = [const.tile([128, w], mm_dt, name=f"xt{s}", tag=f"xt{s}")
                 for s in range(csub)]

        def load_wpt(s):
            nc_eng = nc.gpsimd if s < 6 else nc.scalar
            nc_eng.dma_start(wpt_sb[s][:], wpt_d[128 * s:128 * (s + 1), :])

        def load_xt(s, ch, eng):
            eng.dma_start(
                xt_sb[s][:, chunk * ch:chunk * (ch + 1)],
                xt_d[128 * s:128 * (s + 1), chunk * ch:chunk * (ch + 1)],
            )

        ident_sb = const.tile([128, 128], mm_dt, name="ident", tag="ident")
        nc.scalar.dma_start(ident_sb[:], ident_d[:])
        for s in range(6):
            load_wpt(s)
        for s in range(6):
            load_xt(s, 0, nc.sync)
        load_wpt(6)
        load_xt(6, 0, nc.scalar)
        load_wpt(7)
        load_xt(7, 0, nc.scalar)
        for ch in range(1, nch):
            for s in range(csub):
                load_xt(s, ch, nc.sync)
        mblk_sb = const.tile([128, NPAIR * 128], mm_dt, name="mblk", tag="mblk")
        for p in range(NPAIR):
            nc.scalar.dma_start(mblk_sb[:, 128 * p:128 * (p + 1)], mblk_d[p])
        wm2_sb = []
        for p in range(NPAIR):
            t = const.tile([128, C], mm_dt, name=f"wm2_{p}", tag=f"wm2_{p}")
            nc.gpsimd.dma_start(t[:], wm2_d[p])
            wm2_sb.append(t)
        triu4_sb = const.tile([128, 512], F32, name="triu4", tag="triu4")
        nc.gpsimd.dma_start(triu4_sb[:], triu4_d[:])
        blkd4_sb = const.tile([128, 512], F32, name="blkd4", tag="blkd4")
        nc.gpsimd.dma_start(blkd4_sb[:], blkd4_d[:])

        # ---- persistent intermediates (bf16) ----
        # q_nat: natural layout [w, k] -- w-tile i occupies cols [512i, 512i+512),
        #        inside which head h (0..7) owns cols [64h, 64h+64).
        q_nat = persist.tile([128, nw * GK], mm_dt, name="q_nat", tag="q_nat")
        # qt/at: transposed layout per pair p: cols [p*w, (p+1)*w); partitions
        #        0-63 = head 2p's K dims, 64-127 = head 2p+1's.
        qt_sb = persist.tile([128, NPAIR * w], mm_dt, name="qt_sb", tag="qt_sb")
        at_sb = persist.tile([128, NPAIR * w], mm_dt, name="at_sb", tag="at_sb")
        # per-i blockdiag(KV_a, KV_b) lhsT tiles for the U-main matmuls;
        # layout [i][p]: col block (i*NPAIR+p)*128
        kv_sb = persist.tile([128, nw * NPAIR * 128], mm_dt, name="kv_sb",
                             tag="kv_sb")
        # pre-masked gram terms, layout [i][p] like a [128, 512] row per i
        gt_all = persist.tile([128, max(nw - 1, 1) * NPAIR * 128], mm_dt,
                              name="gt_all", tag="gt_all")
        st_all = persist.tile([128, nw * NPAIR * 256], mm_dt, name="st_all",
                              tag="st_all")
        ut_all = persist.tile([128, nw * NPAIR * 128], mm_dt, name="ut_all",
                              tag="ut_all")

        # ============ phase 1: projection + transposes + gram + C ============
        with tc.tile_pool(name="psA", bufs=3, space="PSUM") as psA, \
                tc.tile_pool(name="psT", bufs=2, space="PSUM") as psT, \
                tc.tile_pool(name="psG", bufs=2, space="PSUM") as psG:

            def emit_A(i):
                ps = psA.tile([128, GK], F32, name="projnat", tag="projnat")
                for s in range(csub):
                    nc.tensor.matmul(
                        ps[:],
                        lhsT=xt_sb[s][:, 128 * i:128 * (i + 1)],
                        rhs=wpt_sb[s][:],
                        start=(s == 0),
                        stop=(s == csub - 1),
                    )
                nc.vector.tensor_copy(q_nat[:, GK * i:GK * (i + 1)], ps[:])

            def emit_T(i):
                # transpose the 4 pair-blocks of q_nat tile i into one psum
                # tile, then one scalar copy into the strided qt_sb layout
                # ([128 part, pair (stride w), 128 w-cols])
                ps = psT.tile([128, 512], mm_dt, name="qtT", tag="qtT")
                for p in range(NPAIR):
                    nc.tensor.transpose(
                        ps[:, 128 * p:128 * (p + 1)],
                        q_nat[:, GK * i + 128 * p:GK * i + 128 * (p + 1)],
                        ident_sb[:],
                    )
                qt_view = qt_sb[:].rearrange(
                    "part (n wdim) -> part n wdim", n=NPAIR
                )[:, :, 128 * i:128 * (i + 1)]
                nc.scalar.copy(qt_view, ps[:])

            def emit_G(i):
                # gram term for w-tile i (pair-stacked), masked on drain
                ps = psG.tile([128, NPAIR * 128], F32, name="gterm", tag="gterm")
                for p in range(NPAIR):
                    qp = q_nat[:, GK * i + 128 * p:GK * i + 128 * (p + 1)]
                    nc.tensor.matmul(
                        ps[:, 128 * p:128 * (p + 1)],
                        lhsT=qp, rhs=qp,
                        start=(p == 0),
                        stop=(p == NPAIR - 1),
                    )
                # masked drain straight to bf16 gram term (kills the
                # serial blockdiag-mul chain); GpSimd can't touch PSUM,
                # so this lives on DVE
                nc.vector.tensor_mul(
                    gt_all[:, i * NPAIR * 128:(i + 1) * NPAIR * 128],
                    ps[:], blkd4_sb[:],
                )

            def emit_C(ch):
                for p in range(NPAIR):
                    ps = psA.tile([128, chunk], F32, name="atps", tag="projnat")
                    nc.tensor.matmul(
                        ps[:],
                        lhsT=mblk_sb[:, 128 * p:128 * (p + 1)],
                        rhs=qt_sb[:, p * w + chunk * ch:p * w + chunk * (ch + 1)],
                        start=True,
                        stop=True,
                    )
                    nc.scalar.copy(
                        at_sb[:, p * w + chunk * ch:p * w + chunk * (ch + 1)],
                        ps[:],
                    )

            # T/G trail A by TWO iterations: the transposes' weight loads wait
            # on the vector A-drain, which lags the PE by ~1 iteration -- at
            # distance 2 the drain has always landed (no ldweights stalls)
            for i in range(nw):
                emit_A(i)
                if i >= 2:
                    emit_T(i - 2)
                    emit_G(i - 2)
                # one extra iteration after T(4ch+3) so the C matmuls never
                # wait on the scalar qt copies
                if i >= 7 and i % 4 == 3:
                    emit_C(i // 4 - 1)
            if nw >= 2:
                emit_T(nw - 2)
                if nw - 2 < nw - 1:
                    emit_G(nw - 2)
            emit_T(nw - 1)
            emit_C(nch - 1)

        # ---- KV prefix: 4 independent per-pair bf16 add chains ----
        # kv[p, 0] unused (i=0 has no main term); zero not required.
        # All-SBUF work, so it all goes to GpSimd (which can't touch PSUM
        # and has nothing else to do), keeping DVE free for PSUM drains.
        chain_eng = [nc.gpsimd, nc.gpsimd, nc.gpsimd, nc.gpsimd]
        for p in range(NPAIR):
            eng = chain_eng[p]
            # kv[p,1] = gt[0,p]
            eng.tensor_copy(
                kv_sb[:, (p * nw + 1) * 128:(p * nw + 1) * 128 + 128],
                gt_all[:, 0 * NPAIR * 128 + 128 * p:0 * NPAIR * 128 + 128 * (p + 1)],
            )
            for i in range(2, nw):
                eng.tensor_add(
                    kv_sb[:, (p * nw + i) * 128:(p * nw + i) * 128 + 128],
                    kv_sb[:, (p * nw + i - 1) * 128:(p * nw + i - 1) * 128 + 128],
                    gt_all[:, (i - 1) * NPAIR * 128 + 128 * p:(i - 1) * NPAIR * 128 + 128 * (p + 1)],
                )

        # ============ phase 2/3: D1a + D1b + mixer, interleaved per i ========
        # psM=3: mixer cm chunks then reuse a buffer drained 1.5 iterations
        # earlier instead of waiting on cm1(i-1)'s late drain; psS=3 still
        # covers the D1a lead since the st masks land mid-iteration
        with tc.tile_pool(name="psS", bufs=3, space="PSUM") as psS, \
                tc.tile_pool(name="psU", bufs=2, space="PSUM") as psU, \
                tc.tile_pool(name="psM", bufs=3, space="PSUM") as psM, \
                tc.tile_pool(name="outp", bufs=3) as outp:

            def emit_D1a(i):
                # all 8 diagonal blocks S_ii^T = Q_i @ A_i^T of tile i.
                # The h=0 / h=1 matmuls of a pair use contraction row groups
                # [0:64) / [64:128) and can execute CONCURRENTLY in the PE
                # array, so they must land in different PSUM banks: batch by h
                # (tile h holds that row-group's block for all 4 pairs).
                for h in range(2):
                    ps = psS.tile([128, 512], F32, name="st", tag="st")
                    for p in range(NPAIR):
                        nc.tensor.matmul(
                            ps[:, 128 * p:128 * (p + 1)],
                            lhsT=qt_sb[64 * h:64 * (h + 1),
                                       p * w + 128 * i:p * w + 128 * (i + 1)],
                            rhs=at_sb[64 * h:64 * (h + 1),
                                      p * w + 128 * i:p * w + 128 * (i + 1)],
                            start=(p == 0),
                            stop=(p == NPAIR - 1),
                        )
                    # masked drain into the strided st_all layout
                    # (cols i*1024 + p*256 + h*128)
                    dst = st_all[:, i * NPAIR * 256:(i + 1) * NPAIR * 256].rearrange(
                        "part (p two) -> part p two", p=NPAIR
                    )[:, :, 128 * h:128 * (h + 1)]
                    nc.vector.tensor_mul(dst, ps[:], triu4_sb[:])

            def emit_D1b(i):
                # UT for all 4 pairs of tile i in one [128, 512] psum tile.
                # Zero regions are per-partition 2KB rows, so the full-width
                # kv matmuls carry the visible start/stop bookkeeping; the
                # partition-split diag matmuls are inexpressible to the sim's
                # flat group tracker (HW has_written bits are per partition)
                # and use skip_group_check, with start=True only on the first
                # write to each partition-row range (i==0, p==0).
                ps = psU.tile([128, 512], F32, name="ut", tag="ut")
                for p in range(NPAIR):
                    st0 = i * NPAIR * 256 + 256 * p
                    if i > 0:
                        nc.tensor.matmul(
                            ps[:, 128 * p:128 * (p + 1)],
                            lhsT=kv_sb[:, (p * nw + i) * 128:(p * nw + i) * 128 + 128],
                            rhs=at_sb[:, p * w + 128 * i:p * w + 128 * (i + 1)],
                            start=(p == 0),
                            stop=(p == NPAIR - 1),
                        )
                    for h in range(2):
                        nc.tensor.matmul(
                            ps[64 * h:64 * (h + 1), 128 * p:128 * (p + 1)],
                            lhsT=q_nat[:, GK * i + 128 * p + 64 * h:
                                       GK * i + 128 * p + 64 * (h + 1)],
                            rhs=st_all[:, st0 + 128 * h:st0 + 128 * (h + 1)],
                            start=(i == 0 and p == 0),
                            stop=(i == 0 and p == NPAIR - 1 and h == 1),
                            skip_group_check=True,
                        )
                nc.scalar.copy(
                    ut_all[:, i * NPAIR * 128:(i + 1) * NPAIR * 128], ps[:]
                )

            def emit_mixer(i):
                # bf16 output tile; each 512-chunk DMAs out right after its
                # drain so the final chunk is the only tail exposure
                out_sb = outp.tile([128, C], mm_dt, name="out_sb", tag="out_sb")
                for cm in range(C // 512):
                    mx = psM.tile([128, 512], F32, name="mx", tag="mx")
                    for p in range(NPAIR):
                        nc.tensor.matmul(
                            mx[:],
                            lhsT=ut_all[:, (i * NPAIR + p) * 128:
                                        (i * NPAIR + p) * 128 + 128],
                            rhs=wm2_sb[p][:, 512 * cm:512 * (cm + 1)],
                            start=(p == 0),
                            stop=(p == NPAIR - 1),
                        )
                    if cm == 0:
                        nc.vector.tensor_copy(out_sb[:, 512 * cm:512 * (cm + 1)], mx[:])
                    else:
                        nc.scalar.copy(out_sb[:, 512 * cm:512 * (cm + 1)], mx[:])
                    nc.sync.dma_start(
                        out_d[128 * i:128 * (i + 1), 512 * cm:512 * (cm + 1)],
                        out_sb[:, 512 * cm:512 * (cm + 1)],
                    )

            # pipeline: D1a runs two tiles ahead and mixer one tile BEHIND,
            # so every consumer is a full iteration (~5us) downstream of the
            # drain that feeds it -- the PE never stalls on the ut drain
            # (scalar) or the st masks (vector)
            emit_D1a(0)
            emit_D1a(1)
            for i in range(nw):
                emit_D1b(i)
                if i + 2 < nw:
                    emit_D1a(i + 2)
                if i > 0:
                    emit_mixer(i - 1)
            emit_mixer(nw - 1)

    nc.finalize()
    return nc


def _get_nc(w=W):
    if w not in _NC_CACHE:
        _NC_CACHE[w] = build_nc(w)
    return _NC_CACHE[w]


def make_in_maps(x, wp, pm, tf, wm, w=W):
    """Host-side shard prep: per-core input dict list (cores c: b=c%4, g=c//4)."""
    bf = ml_dtypes.bfloat16
    metric = np.einsum("nij,nkj->nik", pm, pm) / np.sqrt(np.float32(K))
    # Wm2_n = T_n @ W_mixer[:, nK:(n+1)K]^T : [K, C]
    wm2 = np.stack([tf[n] @ wm[:, n * K:(n + 1) * K].T for n in range(NHEADS)])

    tri = np.triu(np.ones((128, 128), np.float32))
    triu4 = np.tile(tri, (1, 4)).astype(np.float32)
    blkd = np.zeros((128, 128), np.float32)
    blkd[:64, :64] = 1.0
    blkd[64:, 64:] = 1.0
    blkd4 = np.tile(blkd, (1, 4)).astype(np.float32)
    ident = np.eye(128, dtype=np.float32)

    in_maps = []
    for c in range(8):
        b, g = c % 4, c // 4
        xt = np.ascontiguousarray(x[b][:w].T).astype(bf)                    # [C, w]
        wpt = np.ascontiguousarray(wp[GK * g:GK * (g + 1), :].T).astype(bf)  # [C, GK]
        mblk = np.zeros((NPAIR, 128, 128), np.float32)
        wm2c = np.zeros((NPAIR, 128, C), np.float32)
        for p in range(NPAIR):
            ha, hb = HPG * g + 2 * p, HPG * g + 2 * p + 1
            mblk[p, :64, :64] = metric[ha]
            mblk[p, 64:, 64:] = metric[hb]
            wm2c[p, :64, :] = wm2[ha]
            wm2c[p, 64:, :] = wm2[hb]
        in_maps.append({
            "xt": xt,
            "wpt": wpt,
            "mblk": mblk.astype(bf),
            "wm2": wm2c.astype(bf),
            "triu4": triu4,
            "blkd4": blkd4,
            "ident": ident.astype(bf),
        })
    return in_maps


def _ensure_ntff_hook():
    """The agent image lacks antenv.axon_hooks; synthesize it and register the
    ctypes NTFF profile hook from trn_agent_boot so trace=True works."""
    try:
        from antenv.axon_hooks import get_axon_ntff_profile_hook  # noqa: F401
        return
    except ImportError:
        pass
    import types

    import antenv

    mod = types.ModuleType("antenv.axon_hooks")
    _box = {}
    mod.set_axon_ntff_profile_hook = lambda h: _box.__setitem__("h", h)
    mod.get_axon_ntff_profile_hook = lambda: _box.get("h")
    sys.modules["antenv.axon_hooks"] = mod
    antenv.axon_hooks = mod
    try:
        from trn_agent_boot.trn_boot import _ntff_profile_via_ctypes

        h = _ntff_profile_via_ctypes("/opt/axon/libaxon_pjrt.so")
        if h is not None:
            mod.set_axon_ntff_profile_hook(h)
    except Exception as e:  # profiling degrades, run still works
        print(f"ntff hook setup failed: {e}", file=sys.stderr)


def kernel(**inputs):
    global LAST_RESULTS
    x = np.asarray(inputs["in_sequence_bwc"], np.float32)
    wp = np.asarray(inputs["W_proj"], np.float32)
    pm = np.asarray(inputs["pre_metric_nkk"], np.float32)
    tf = np.asarray(inputs["transforms_nkk"], np.float32)
    wm = np.asarray(inputs["W_mixer"], np.float32)

    in_maps = make_in_maps(x, wp, pm, tf, wm)
    nc = _get_nc()
    trace = bool(int(os.environ.get("KERNEL_TRACE", "0")))
    if trace:
        _ensure_ntff_hook()
    res = run_bass_kernel_spmd(nc, in_maps, list(range(8)), trace=trace)
    LAST_RESULTS = res
    outs = [np.asarray(r["out"], np.float32) for r in res.results]
    full = np.empty((B, W, C), np.float32)
    for b in range(B):
        full[b] = outs[b] + outs[4 + b]
    return full



# revision 4
# speedup vs baseline: 1.1399x; 1.1399x over previous
"""Trainium2 Bass kernel for nn_MetricSelfAttention.

Reference computation (B=4, W=2048, C=1024, N=16 heads, K=64):
    metric_n = P_n @ P_n^T                  (per-head bilinear form)
    proj = X @ W_proj^T ; split into per-head Q_n [W, K]
    S_n = tril(Q_n M_n Q_n^T) / sqrt(K)     (multiplicative causal mask, no softmax)
    U_n = S_n @ Q_n
    out = concat_n(U_n @ T_n) @ W_mixer^T

Device algorithm (per core; 8 cores = 4 batches x 2 head-groups of 8 heads):
  Host folds:  M'_n = P_n P_n^T / sqrt(K),  Wm2_n = T_n @ W_mixer[:, nK:(n+1)K]^T
  so that out_partial = sum_n U_n @ Wm2_n with U_n = tril(Q_n M'_n Q_n^T) @ Q_n.

  Causal decomposition (block row i of 128):
    U_i = A_i @ KV_i + tril(A_i Q_i^T) @ Q_i,   A = Q M',  KV_i = sum_{j<i} Q_j^T Q_j

  v2 structure (126us -> target ~95us), trace-driven:
   - qt/at/gt/st are PER-TILE tiles: the tile framework coarsens strided
     (rearranged) writes to whole-tile dependencies, which made phase 2
     wait for the LAST phase-1 scalar copy (~5us transition bubble) and
     stalled the gpsimd KV chains until ~56us
   - emit_C is per-tile (N=128 x 4 pairs) trailing the transposes inside
     the phase-1 loop, so no end-of-phase C(3) drain serialization
   - KV prefix adds are emitted i-MAJOR inside the phase-1 loop (entry
     e=i-2 at loop i), pacing gpsimd with the grams instead of running
     four p-major chains that finish at ~82us
   - emit_D1b issues all 4 main matmuls FIRST, then the 8 diag matmuls:
     diag(p) accumulates onto the psum region main(p) wrote, and at
     distance <3 slots the psum RMW hazard serialized each pair (~328ns
     vs ~110ns achievable)
   - startup: the 16 critical first loads (xt ch0 + wpt, 2MB) alternate
     sync/gpsimd queues in consumption order; ident/mblk go on scalar;
     blkd/triu/wm2 load behind the critical set on gpsimd
"""

import os
import sys

import numpy as np
import ml_dtypes

if "/opt/trn_rl_repo" not in sys.path:
    sys.path.insert(0, "/opt/trn_rl_repo")

import concourse.bass as bass
import concourse.tile as tile
from concourse import bacc, mybir
from concourse.bass_utils import run_bass_kernel_spmd

BF16 = mybir.dt.bfloat16
F32 = mybir.dt.float32

B, W, C, NHEADS, K = 4, 2048, 1024, 16, 64
HPG = 8          # heads per group (per core)
NPAIR = 4        # head pairs per core
GK = HPG * K     # 512: head-group projection width

_NC_CACHE = {}
LAST_RESULTS = None  # for test.py introspection (exec_time_ns etc.)


def build_nc(w=W, mm_dt=BF16):
    """Build the per-core Bass program. Parameterized by sequence length for
    small-scale simulator testing."""
    nw = w // 128           # number of 128-row w-tiles
    csub = C // 128         # 8 contraction subtiles for the projections
    chunk = min(512, w)
    nch = w // chunk        # 512-wide chunks of the sequence dim

    nc = bacc.Bacc()
    xt_d = nc.declare_dram_parameter("xt", [C, w], mm_dt, isOutput=False)
    wpt_d = nc.declare_dram_parameter("wpt", [C, GK], mm_dt, isOutput=False)
    mblk_d = nc.declare_dram_parameter("mblk", [NPAIR, 128, 128], mm_dt, isOutput=False)
    wm2_d = nc.declare_dram_parameter("wm2", [NPAIR, 128, C], mm_dt, isOutput=False)
    triu4_d = nc.declare_dram_parameter("triu4", [128, 512], F32, isOutput=False)
    blkd4_d = nc.declare_dram_parameter("blkd4", [128, 512], F32, isOutput=False)
    ident_d = nc.declare_dram_parameter("ident", [128, 128], mm_dt, isOutput=False)
    out_d = nc.declare_dram_parameter("out", [w, C], mm_dt, isOutput=True)

    from contextlib import ExitStack

    with tile.TileContext(nc) as tc, ExitStack() as ctx:
        const = ctx.enter_context(tc.tile_pool(name="const", bufs=1))
        persist = ctx.enter_context(tc.tile_pool(name="persist", bufs=1))

        # ---- constant / input loads ----
        # A(0) consumes subtiles in s order; xt-ch0/wpt loads alternate the
        # sync and gpsimd queues so both operands of sub-matmul s land at
        # roughly the same time.  Non-critical consts (blkd/triu/wm2) queue
        # behind them; ident/mblk ride the otherwise-idle scalar queue.
        wpt_sb = [const.tile([128, GK], mm_dt, name=f"wpt{s}", tag=f"wpt{s}")
                  for s in range(csub)]
        xt_sb = [const.tile([128, w], mm_dt, name=f"xt{s}", tag=f"xt{s}")
                 for s in range(csub)]

        def load_wpt(s, eng):
            eng.dma_start(wpt_sb[s][:], wpt_d[128 * s:128 * (s + 1), :])

        def load_xt(s, ch, eng):
            eng.dma_start(
                xt_sb[s][:, chunk * ch:chunk * (ch + 1)],
                xt_d[128 * s:128 * (s + 1), chunk * ch:chunk * (ch + 1)],
            )

        for s in range(csub):
            if s % 2 == 0:
                load_xt(s, 0, nc.sync)
                load_wpt(s, nc.gpsimd)
            else:
                load_xt(s, 0, nc.gpsimd)
                load_wpt(s, nc.sync)
        ident_sb = const.tile([128, 128], mm_dt, name="ident", tag="ident")
        nc.scalar.dma_start(ident_sb[:], ident_d[:])
        mblk_sb = const.tile([128, NPAIR * 128], mm_dt, name="mblk", tag="mblk")
        for p in range(NPAIR):
            nc.scalar.dma_start(mblk_sb[:, 128 * p:128 * (p + 1)], mblk_d[p])
        blkd4_sb = const.tile([128, 512], F32, name="blkd4", tag="blkd4")
        nc.gpsimd.dma_start(blkd4_sb[:], blkd4_d[:])
        for ch in range(1, nch):
            for s in range(csub):
                load_xt(s, ch, nc.sync)
        triu4_sb = const.tile([128, 512], F32, name="triu4", tag="triu4")
        nc.gpsimd.dma_start(triu4_sb[:], triu4_d[:])
        wm2_sb = []
        for p in range(NPAIR):
            t = const.tile([128, C], mm_dt, name=f"wm2_{p}", tag=f"wm2_{p}")
            nc.gpsimd.dma_start(t[:], wm2_d[p])
            wm2_sb.append(t)

        # ---- persistent intermediates (bf16) ----
        # q_nat: natural layout [w, k] -- w-tile i occupies cols [512i, 512i+512),
        #        inside which head h (0..7) owns cols [64h, 64h+64).
        q_nat = persist.tile([128, nw * GK], mm_dt, name="q_nat", tag="q_nat")
        # qt/at: PER-TILE transposed layout: pair p at cols [128p, 128p+128),
        #        partitions 0-63 = head 2p's K dims, 64-127 = head 2p+1's.
        qt_t = [persist.tile([128, 512], mm_dt, name=f"qt{i}", tag=f"qt{i}")
                for i in range(nw)]
        at_t = [persist.tile([128, 512], mm_dt, name=f"at{i}", tag=f"at{i}")
                for i in range(nw)]
        # pre-masked gram terms, per-tile, pair p at cols [128p, 128p+128)
        gt_t = [persist.tile([128, 512], mm_dt, name=f"gt{i}", tag=f"gt{i}")
                for i in range(max(nw - 1, 1))]
        # per-i blockdiag(KV_a, KV_b) lhsT tiles for the U-main matmuls;
        # layout [p][i]: col block (p*nw+i)*128
        kv_sb = persist.tile([128, nw * NPAIR * 128], mm_dt, name="kv_sb",
                             tag="kv_sb")
        # masked S^T blocks, per-tile, layout [p][h]: col 256p+128h
        st_t = [persist.tile([128, NPAIR * 256], mm_dt, name=f"st{i}",
                             tag=f"st{i}") for i in range(nw)]
        ut_all = persist.tile([128, nw * NPAIR * 128], mm_dt, name="ut_all",
                              tag="ut_all")

        # ============ phase 1: projection + transposes + gram + C + KV ======
        with tc.tile_pool(name="psA", bufs=2, space="PSUM") as psA, \
                tc.tile_pool(name="psT", bufs=2, space="PSUM") as psT, \
                tc.tile_pool(name="psG", bufs=2, space="PSUM") as psG, \
                tc.tile_pool(name="psC", bufs=2, space="PSUM") as psC:

            def emit_A(i):
                ps = psA.tile([128, GK], F32, name="projnat", tag="projnat")
                for s in range(csub):
                    nc.tensor.matmul(
                        ps[:],
                        lhsT=xt_sb[s][:, 128 * i:128 * (i + 1)],
                        rhs=wpt_sb[s][:],
                        start=(s == 0),
                        stop=(s == csub - 1),
                    )
                nc.vector.tensor_copy(q_nat[:, GK * i:GK * (i + 1)], ps[:])

            def emit_T(i):
                # transpose the 4 pair-blocks of q_nat tile i into one psum
                # tile; the psum layout IS the per-tile qt layout, so the
                # scalar drain is a plain contiguous copy
                ps = psT.tile([128, 512], mm_dt, name="qtT", tag="qtT")
                for p in range(NPAIR):
                    nc.tensor.transpose(
                        ps[:, 128 * p:128 * (p + 1)],
                        q_nat[:, GK * i + 128 * p:GK * i + 128 * (p + 1)],
                        ident_sb[:],
                    )
                nc.scalar.copy(qt_t[i][:], ps[:])

            def emit_G(i):
                # gram term for w-tile i (pair-stacked), masked on drain
                ps = psG.tile([128, NPAIR * 128], F32, name="gterm", tag="gterm")
                for p in range(NPAIR):
                    qp = q_nat[:, GK * i + 128 * p:GK * i + 128 * (p + 1)]
                    nc.tensor.matmul(
                        ps[:, 128 * p:128 * (p + 1)],
                        lhsT=qp, rhs=qp,
                        start=(p == 0),
                        stop=(p == NPAIR - 1),
                    )
                nc.vector.tensor_mul(gt_t[i][:], ps[:], blkd4_sb[:])

            def emit_C(i):
                # at tile i = M'_blk @ qt tile i, per pair (N=128 each)
                ps = psC.tile([128, 512], F32, name="atps", tag="atps")
                for p in range(NPAIR):
                    nc.tensor.matmul(
                        ps[:, 128 * p:128 * (p + 1)],
                        lhsT=mblk_sb[:, 128 * p:128 * (p + 1)],
                        rhs=qt_t[i][:, 128 * p:128 * (p + 1)],
                        start=(p == 0),
                        stop=(p == NPAIR - 1),
                    )
                nc.scalar.copy(at_t[i][:], ps[:])

            def emit_KV(e):
                # kv[p, e] = kv[p, e-1] + gt[e-1][p]  (i-major: all 4 pairs)
                for p in range(NPAIR):
                    dst = kv_sb[:, (p * nw + e) * 128:(p * nw + e) * 128 + 128]
                    gsl = gt_t[e - 1][:, 128 * p:128 * (p + 1)]
                    if e == 1:
                        nc.gpsimd.tensor_copy(dst, gsl)
                    else:
                        prv = kv_sb[:, (p * nw + e - 1) * 128:
                                    (p * nw + e - 1) * 128 + 128]
                        nc.gpsimd.tensor_add(dst, prv, gsl)

            # T/G trail A by TWO iterations (q_nat vector drain lag); C and
            # the KV adds trail one more (qt scalar copy / gt vector drain)
            for i in range(nw):
                emit_A(i)
                if i >= 2:
                    emit_T(i - 2)
                    emit_G(i - 2)
                if i >= 3:
                    emit_C(i - 3)
                    if i - 2 < nw:
                        emit_KV(i - 2)
            if nw >= 2:
                emit_T(nw - 2)
                emit_G(nw - 2)
            emit_T(nw - 1)
            for i in range(max(nw - 3, 0), nw):
                emit_C(i)
            for e in range(max(nw - 2, 1), nw):
                emit_KV(e)

        # ============ phase 2/3: D1a + D1b + mixer, interleaved per i ========
        with tc.tile_pool(name="psS", bufs=3, space="PSUM") as psS, \
                tc.tile_pool(name="psU", bufs=2, space="PSUM") as psU, \
                tc.tile_pool(name="psM", bufs=3, space="PSUM") as psM, \
                tc.tile_pool(name="outp", bufs=3) as outp:

            def emit_D1a(i):
                # all 8 diagonal blocks S_ii^T = Q_i @ A_i^T of tile i.
                # The h=0 / h=1 matmuls of a pair use contraction row groups
                # [0:64) / [64:128) and execute CONCURRENTLY in the PE array
                # (separate psum tiles per h).
                for h in range(2):
                    ps = psS.tile([128, 512], F32, name="st", tag="st")
                    for p in range(NPAIR):
                        nc.tensor.matmul(
                            ps[:, 128 * p:128 * (p + 1)],
                            lhsT=qt_t[i][64 * h:64 * (h + 1),
                                         128 * p:128 * (p + 1)],
                            rhs=at_t[i][64 * h:64 * (h + 1),
                                        128 * p:128 * (p + 1)],
                            start=(p == 0),
                            stop=(p == NPAIR - 1),
                        )
                    # masked drain into st tile i (cols 256p + 128h)
                    dst = st_t[i][:].rearrange(
                        "part (p two) -> part p two", p=NPAIR
                    )[:, :, 128 * h:128 * (h + 1)]
                    nc.vector.tensor_mul(dst, ps[:], triu4_sb[:])

            def emit_D1b(i):
                # UT for all 4 pairs of tile i in one [128, 512] psum tile.
                # All 4 main matmuls go FIRST (disjoint col blocks), then the
                # 8 diag matmuls: diag(p) accumulates onto main(p)'s region,
                # and at issue distance >=3 slots the psum RMW hazard is
                # fully hidden.  The partition-split diag matmuls are
                # inexpressible to the sim's flat group tracker (HW
                # has_written bits are per partition) and use
                # skip_group_check, with start=True only on the first write
                # to each partition-row range (i==0, p==0).
                ps = psU.tile([128, 512], F32, name="ut", tag="ut")
                if i > 0:
                    for p in range(NPAIR):
                        nc.tensor.matmul(
                            ps[:, 128 * p:128 * (p + 1)],
                            lhsT=kv_sb[:, (p * nw + i) * 128:
                                       (p * nw + i) * 128 + 128],
                            rhs=at_t[i][:, 128 * p:128 * (p + 1)],
                            start=(p == 0),
                            stop=(p == NPAIR - 1),
                        )
                for p in range(NPAIR):
                    for h in range(2):
                        nc.tensor.matmul(
                            ps[64 * h:64 * (h + 1), 128 * p:128 * (p + 1)],
                            lhsT=q_nat[:, GK * i + 128 * p + 64 * h:
                                       GK * i + 128 * p + 64 * (h + 1)],
                            rhs=st_t[i][:, 256 * p + 128 * h:
                                        256 * p + 128 * (h + 1)],
                            start=(i == 0 and p == 0),
                            stop=(i == 0 and p == NPAIR - 1 and h == 1),
                            skip_group_check=True,
                        )
                nc.scalar.copy(
                    ut_all[:, i * NPAIR * 128:(i + 1) * NPAIR * 128], ps[:]
                )

            def emit_mixer(i):
                # bf16 output tile; each 512-chunk DMAs out right after its
                # drain so the final chunk is the only tail exposure
                out_sb = outp.tile([128, C], mm_dt, name="out_sb", tag="out_sb")
                for cm in range(C // 512):
                    mx = psM.tile([128, 512], F32, name="mx", tag="mx")
                    for p in range(NPAIR):
                        nc.tensor.matmul(
                            mx[:],
                            lhsT=ut_all[:, (i * NPAIR + p) * 128:
                                        (i * NPAIR + p) * 128 + 128],
                            rhs=wm2_sb[p][:, 512 * cm:512 * (cm + 1)],
                            start=(p == 0),
                            stop=(p == NPAIR - 1),
                        )
                    if cm == 0:
                        nc.vector.tensor_copy(out_sb[:, 512 * cm:512 * (cm + 1)], mx[:])
                    else:
                        nc.scalar.copy(out_sb[:, 512 * cm:512 * (cm + 1)], mx[:])
                    nc.sync.dma_start(
                        out_d[128 * i:128 * (i + 1), 512 * cm:512 * (cm + 1)],
                        out_sb[:, 512 * cm:512 * (cm + 1)],
                    )

            # pipeline: D1a runs two tiles ahead and mixer one tile BEHIND,
            # so every consumer is a full iteration downstream of the drain
            # that feeds it
            emit_D1a(0)
            emit_D1a(1)
            for i in range(nw):
                emit_D1b(i)
                if i + 2 < nw:
                    emit_D1a(i + 2)
                if i > 0:
                    emit_mixer(i - 1)
            emit_mixer(nw - 1)

    nc.finalize()
    return nc


def _get_nc(w=W):
    if w not in _NC_CACHE:
        _NC_CACHE[w] = build_nc(w)
    return _NC_CACHE[w]


def make_in_maps(x, wp, pm, tf, wm, w=W):
    """Host-side shard prep: per-core input dict list (cores c: b=c%4, g=c//4)."""
    bf = ml_dtypes.bfloat16
    metric = np.einsum("nij,nkj->nik", pm, pm) / np.sqrt(np.float32(K))
    # Wm2_n = T_n @ W_mixer[:, nK:(n+1)K]^T : [K, C]
    wm2 = np.stack([tf[n] @ wm[:, n * K:(n + 1) * K].T for n in range(NHEADS)])

    tri = np.triu(np.ones((128, 128), np.float32))
    triu4 = np.tile(tri, (1, 4)).astype(np.float32)
    blkd = np.zeros((128, 128), np.float32)
    blkd[:64, :64] = 1.0
    blkd[64:, 64:] = 1.0
    blkd4 = np.tile(blkd, (1, 4)).astype(np.float32)
    ident = np.eye(128, dtype=np.float32)

    in_maps = []
    for c in range(8):
        b, g = c % 4, c // 4
        xt = np.ascontiguousarray(x[b][:w].T).astype(bf)                    # [C, w]
        wpt = np.ascontiguousarray(wp[GK * g:GK * (g + 1), :].T).astype(bf)  # [C, GK]
        mblk = np.zeros((NPAIR, 128, 128), np.float32)
        wm2c = np.zeros((NPAIR, 128, C), np.float32)
        for p in range(NPAIR):
            ha, hb = HPG * g + 2 * p, HPG * g + 2 * p + 1
            mblk[p, :64, :64] = metric[ha]
            mblk[p, 64:, 64:] = metric[hb]
            wm2c[p, :64, :] = wm2[ha]
            wm2c[p, 64:, :] = wm2[hb]
        in_maps.append({
            "xt": xt,
            "wpt": wpt,
            "mblk": mblk.astype(bf),
            "wm2": wm2c.astype(bf),
            "triu4": triu4,
            "blkd4": blkd4,
            "ident": ident.astype(bf),
        })
    return in_maps


def _ensure_ntff_hook():
    """The agent image lacks antenv.axon_hooks; synthesize it and register the
    ctypes NTFF profile hook from trn_agent_boot so trace=True works."""
    try:
        from antenv.axon_hooks import get_axon_ntff_profile_hook  # noqa: F401
        return
    except ImportError:
        pass
    import types

    import antenv

    mod = types.ModuleType("antenv.axon_hooks")
    _box = {}
    mod.set_axon_ntff_profile_hook = lambda h: _box.__setitem__("h", h)
    mod.get_axon_ntff_profile_hook = lambda: _box.get("h")
    sys.modules["antenv.axon_hooks"] = mod
    antenv.axon_hooks = mod
    try:
        from trn_agent_boot.trn_boot import _ntff_profile_via_ctypes

        h = _ntff_profile_via_ctypes("/opt/axon/libaxon_pjrt.so")
        if h is not None:
            mod.set_axon_ntff_profile_hook(h)
    except Exception as e:  # profiling degrades, run still works
        print(f"ntff hook setup failed: {e}", file=sys.stderr)


def kernel(**inputs):
    global LAST_RESULTS
    x = np.asarray(inputs["in_sequence_bwc"], np.float32)
    wp = np.asarray(inputs["W_proj"], np.float32)
    pm = np.asarray(inputs["pre_metric_nkk"], np.float32)
    tf = np.asarray(inputs["transforms_nkk"], np.float32)
    wm = np.asarray(inputs["W_mixer"], np.float32)

    in_maps = make_in_maps(x, wp, pm, tf, wm)
    nc = _get_nc()
    trace = bool(int(os.environ.get("KERNEL_TRACE", "0")))
    if trace:
        _ensure_ntff_hook()
    res = run_bass_kernel_spmd(nc, in_maps, list(range(8)), trace=trace)
    LAST_RESULTS = res
    outs = [np.asarray(r["out"], np.float32) for r in res.results]
    full = np.empty((B, W, C), np.float32)
    for b in range(B):
        full[b] = outs[b] + outs[4 + b]
    return full
